# revision 23
# baseline (speedup 1.0000x reference)
# BitConvBlock Trainium2 kernel: LayerNorm -> activation int8-quant ->
# ternary weight quant -> conv1d(K=3, pad 1) -> rescale.
#
# Sharding: data-parallel over batch (B=8) across the 8 NeuronCores; every
# core gets one batch element plus replicated W / ln params, computes its
# full [T, C] output slice, host stacks the results.
#
# v2 schedule (post-trace rebalance):
#   - x is read from HBM twice total (prologue stats+xhat+extrema in one
#     sweep with the tile held in SBUF; produce re-reads). W is read once
#     and stashed in SBUF; quantization consumes the stash.
#   - engine balance: ACT does xhat + W abs/sign; DVE does stats, max
#     extrema, W fma, fused RNE rounds, rescale; gpsimd does the min
#     extrema chain + output DMA; PE does only matmuls, wq transposes and
#     small broadcasts; xq transpose via DMA XBAR.
#   - NQ=8 produce/consume chunks so the PE matmul stream starts ~90us in
#     and never starves afterwards.
#
# Exactness: x_q integer in [-127,127], w_q in {-1,0,1}; both exact in
# bf16; every partial conv sum < 2^24 so bf16 matmul + fp32 PSUM accum is
# exact. Rounding uses the fp32 +-1.5*2^23 trick (round-to-nearest-even,
# matches jnp.round).

import numpy as np

import concourse.bacc as bacc
import concourse.bass as bass
import concourse.mybir as mybir
import concourse.tile as tile
from concourse.bass_utils import run_bass_kernel_spmd
from concourse.masks import make_identity

F32 = mybir.dt.float32
BF16 = mybir.dt.bfloat16
AX = mybir.AxisListType
OP = mybir.AluOpType
AF = mybir.ActivationFunctionType

QP = 127.0
EPS_LN = 1e-5
EPS_CLAMP = 1e-5
RC = 1.5 * 2.0**23  # fp32 round-to-nearest-even magic constant
N_CORES = 8
KW = 3  # conv kernel width


def build_kernel(T, C, beta_zero, n_cores=N_CORES):
    """Build and compile the per-core Bass program for x:[T,C] W:[C,C,3]."""
    assert T % 128 == 0 and C % 128 == 0
    NT = T // 128            # time tiles
    NCC = C // 128           # channel chunks of 128
    OSL = min(512, C)        # output-channel slab (one PSUM bank)
    NH = C // OSL            # slabs per tile
    TQ = min(512, T)         # produce/consume chunk along T
    NQ = T // TQ
    NTQ = TQ // 128          # time tiles per chunk / stat group
    SUB = min(512, C)        # bn_stats subgroup
    NS = C // SUB
    XPAD = 16                # left pad in xqT so xbar writes stay 32B-aligned
    W_COUNT = float(C * C * KW)

    nc = bacc.Bacc("TRN2", target_bir_lowering=False, debug=False,
                   num_devices=n_cores)
    x_d = nc.dram_tensor("x", [T, C], F32, kind="ExternalInput")
    g_d = nc.dram_tensor("ln_gamma", [C], F32, kind="ExternalInput")
    b_d = nc.dram_tensor("ln_beta", [C], F32, kind="ExternalInput")
    w_d = nc.dram_tensor("W", [C, C, KW], F32, kind="ExternalInput")
    out_d = nc.dram_tensor("out", [T, C], F32, kind="ExternalOutput")

    with tile.TileContext(nc) as tc:
        import contextlib
        with contextlib.ExitStack() as ctx:
            dram = ctx.enter_context(tc.tile_pool(name="dram", bufs=1, space="DRAM"))
            xq_dram = dram.tile([T, C], BF16)

            const = ctx.enter_context(tc.tile_pool(name="const", bufs=1))
            ident = const.tile([128, 128], F32)
            make_identity(nc, ident[:])
            identb = const.tile([128, 128], BF16)
            nc.vector.tensor_copy(identb[:], ident[:])
            ones_col = const.tile([128, 1], F32)
            nc.vector.memset(ones_col[:], 1.0)
            ones_row = const.tile([1, 128], F32)
            nc.vector.memset(ones_row[:], 1.0)
            rcn_col = const.tile([128, 1], F32)
            nc.vector.memset(rcn_col[:], -RC)

            mv_all = const.tile([128, NT, 2], F32)    # per-tile mean/var
            rsig_all = const.tile([128, NT], F32)
            nmr_all = const.tile([128, NT], F32)      # -mu * rsig
            wabs = const.tile([128, NCC * KW], F32)   # per (o-tile,k) |W| sums
            beta_col = const.tile([128, 1], F32)
            binv_col = const.tile([128, 1], F32)

            amx_t = const.tile([128, C], F32)         # running max of xhat
            amn_t = const.tile([128, C], F32)         # running min of xhat

            # big persistent bf16 weight operand
            wqt_all = const.tile([128, KW, NCC, C], BF16)

            # ---------------- working pools -------------------------------
            xin = ctx.enter_context(tc.tile_pool(name="xin", bufs=4))
            xin2 = ctx.enter_context(tc.tile_pool(name="xin2", bufs=2))
            xhat_p = ctx.enter_context(tc.tile_pool(name="xhat", bufs=2))
            xq_p = ctx.enter_context(tc.tile_pool(name="xq", bufs=2))
            wq_p = ctx.enter_context(tc.tile_pool(name="wq", bufs=2))
            yout = ctx.enter_context(tc.tile_pool(name="yout", bufs=2))
            small = ctx.enter_context(tc.tile_pool(name="small", bufs=4))
            st_p = ctx.enter_context(tc.tile_pool(name="st", bufs=2))
            grp_p = ctx.enter_context(tc.tile_pool(name="grp", bufs=3))

            psum_mm = ctx.enter_context(
                tc.tile_pool(name="psum_mm", bufs=6, space="PSUM"))
            psum_ms = ctx.enter_context(
                tc.tile_pool(name="psum_ms", bufs=2, space="PSUM"))

            def ptile():
                return psum_ms.tile([128, 512], F32, tag="ms", name="pms")

            def pbtile():
                return psum_ms.tile([128, 512], BF16, tag="ms", name="pmsb")

            SG = 2                   # stats group size (x tiles held in SBUF)
            NG = NT // SG

            def gtile():
                return grp_p.tile([128, SG], F32, tag="g1", name="gt")

            # =================== prologue =================================
            # W chain: load all of W into an SBUF stash (single HBM read),
            # abs-sum per o-tile -> beta -> fma(u) on DVE -> Sign on ACT ->
            # PE transpose into wqt_all. x chain: single sweep computing
            # bn stats, xhat and the running max/min extrema, holding each
            # x tile in SBUF between its two uses.
            with tc.tile_pool(name="wstash", bufs=1) as stash_p, \
                 tc.tile_pool(name="upool", bufs=1) as u_pool:
                wstash = stash_p.tile([128, NCC, C, KW], F32)

                # W loads first (scalar HWDGE queue) so W lands early.
                for ot in range(NCC):
                    nc.scalar.dma_start(
                        out=wstash[:, ot, :, :],
                        in_=w_d[ot * 128:(ot + 1) * 128, :, :])

                # |W| row-sums for one o-tile (ACT Abs with accumulate)
                def wabs_tile(ot):
                    for k in range(KW):
                        ut = u_pool.tile([128, C], F32, tag="ut", name="ut")
                        nc.scalar.activation(
                            ut[:], wstash[:, ot, :, k], AF.Abs,
                            accum_out=wabs[:, ot * KW + k:ot * KW + k + 1])

                # beta_w = max(mean|W|, eps), replicated to [128,1]
                def beta_chain():
                    wsum = small.tile([128, 1], F32, tag="wsum", name="wsum")
                    nc.vector.reduce_sum(wsum[:], wabs[:], axis=AX.X)
                    ps1 = psum_ms.tile([1, 1], F32, tag="ms", name="ps1")
                    nc.tensor.matmul(ps1[:], ones_col[:], wsum[:], start=True,
                                     stop=True)
                    bsc = small.tile([1, 1], F32, tag="bsc", name="bsc")
                    nc.vector.tensor_scalar(bsc[:], ps1[:], 1.0 / W_COUNT,
                                            EPS_CLAMP, op0=OP.mult, op1=OP.max)
                    psb = psum_ms.tile([128, 1], F32, tag="ms", name="psb")
                    nc.tensor.matmul(psb[:], ones_row[:], bsc[:], start=True,
                                     stop=True)
                    nc.vector.tensor_copy(beta_col[:], psb[:])
                    nc.vector.reciprocal(binv_col[:], beta_col[:])

                # W quantize + transpose, one o-tile at a time.
                def w2_tile(ot):
                    for k in range(KW):
                        ut = u_pool.tile([128, C], F32, tag="ut", name="ut")
                        # u = rne(w/beta) + RC (DVE: mult rounds once, add
                        # to the RC grid rounds-to-nearest-even)
                        nc.vector.tensor_scalar(ut[:], wstash[:, ot, :, k],
                                                binv_col[:], RC,
                                                op0=OP.mult, op1=OP.add)
                        wqk = wq_p.tile([128, C], BF16, tag="wqk", name="wqk")
                        nc.scalar.activation(wqk[:], ut[:], AF.Sign,
                                             bias=rcn_col[:], scale=1.0)
                        for jb2 in range(0, NCC, 4):
                            pb = pbtile()
                            nblk = min(4, NCC - jb2)
                            for b in range(nblk):
                                jb = jb2 + b
                                nc.tensor.transpose(
                                    pb[:, b * 128:(b + 1) * 128],
                                    wqk[:, jb * 128:(jb + 1) * 128],
                                    identb[:])
                            nc.vector.tensor_copy(
                                wqt_all[:, k, jb2:jb2 + nblk,
                                        ot * 128:(ot + 1) * 128],
                                pb[:, 0:nblk * 128])

                # x sweep group g covers tiles [g*SG, (g+1)*SG)
                def x_group(g):
                    xts = []
                    for itq in range(SG):
                        it = g * SG + itq
                        xt = xin.tile([128, C], F32)
                        nc.sync.dma_start(out=xt[:],
                                          in_=x_d[it * 128:(it + 1) * 128, :])
                        st6 = st_p.tile([128, NS, 6], F32)
                        for sb in range(NS):
                            nc.vector.bn_stats(st6[:, sb, :],
                                               xt[:, sb * SUB:(sb + 1) * SUB])
                        nc.vector.bn_aggr(mv_all[:, it, :], st6[:])
                        xts.append(xt)
                    # batched rsqrt + one Newton step for the group
                    gs = slice(g * SG, (g + 1) * SG)
                    ve = gtile()
                    nc.vector.tensor_scalar_add(ve[:], mv_all[:, gs, 1],
                                                EPS_LN)
                    s0 = gtile()
                    nc.scalar.activation(s0[:], ve[:], AF.Sqrt)
                    r0 = gtile()
                    nc.vector.reciprocal(r0[:], s0[:])
                    r2 = gtile()
                    nc.vector.tensor_mul(r2[:], r0[:], r0[:])
                    nc.vector.tensor_mul(r2[:], r2[:], ve[:])
                    nc.vector.tensor_scalar(r2[:], r2[:], -0.5, 1.5,
                                            op0=OP.mult, op1=OP.add)
                    nc.vector.tensor_tensor(rsig_all[:, gs], r0[:], r2[:],
                                            op=OP.mult)
                    mr = gtile()
                    nc.vector.tensor_tensor(mr[:], mv_all[:, gs, 0],
                                            rsig_all[:, gs], op=OP.mult)
                    nc.vector.tensor_scalar_mul(nmr_all[:, gs], mr[:], -1.0)
                    # xhat + extrema from the SBUF-held tiles
                    for itq in range(SG):
                        it = g * SG + itq
                        xh = xhat_p.tile([128, C], F32, tag="xh2", name="xh2")
                        nc.scalar.activation(xh[:], xts[itq][:], AF.Identity,
                                             bias=nmr_all[:, it:it + 1],
                                             scale=rsig_all[:, it:it + 1])
                        if it == 0:
                            nc.vector.tensor_copy(amx_t[:], xh[:])
                            nc.vector.tensor_copy(amn_t[:], xh[:])
                        else:
                            nc.vector.tensor_tensor(amx_t[:], amx_t[:],
                                                    xh[:], op=OP.max)
                            nc.vector.tensor_tensor(amn_t[:], amn_t[:],
                                                    xh[:], op=OP.min)

                # interleave: x groups paced by DMA; W abs / beta / quant
                # slotted between groups to match W DMA arrival, so no
                # engine queue stalls at its head.
                wa_done = 0
                beta_done = False
                w2_done = 0
                for g in range(NG):
                    x_group(g)
                    if g == 0:
                        continue
                    if wa_done < NCC:
                        wabs_tile(wa_done)
                        wa_done += 1
                    elif not beta_done:
                        beta_chain()
                        beta_done = True
                    elif w2_done < NCC:
                        for _ in range(2):
                            if w2_done < NCC:
                                w2_tile(w2_done)
                                w2_done += 1
                while wa_done < NCC:
                    wabs_tile(wa_done)
                    wa_done += 1
                if not beta_done:
                    beta_chain()
                while w2_done < NCC:
                    w2_tile(w2_done)
                    w2_done += 1

            # ============ per-channel scales -> broadcast rows ============
            # broadcast matrices live after the W stash is released
            bcast = ctx.enter_context(tc.tile_pool(name="bcast", bufs=1))
            A_b = bcast.tile([128, C], F32)
            B_b = (bcast.tile([128, C], F32, name="B_b") if not beta_zero
                   else None)
            r_b = bcast.tile([128, C], F32)
            g_mat = small.tile([128, NCC], F32, tag="gmat", name="gmat")
            nc.gpsimd.dma_start(out=g_mat[:],
                                in_=g_d.ap().rearrange("(j p) -> p j", p=128))
            if not beta_zero:
                b_mat = small.tile([128, NCC], F32, tag="bmat", name="bmat")
                nc.gpsimd.dma_start(out=b_mat[:],
                                    in_=b_d.ap().rearrange("(j p) -> p j", p=128))
            m1 = small.tile([128, NCC], F32, tag="m1", name="m1")
            Mx = small.tile([128, NCC], F32, tag="Mx", name="Mx")
            Mn = small.tile([128, NCC], F32, tag="Mn", name="Mn")
            for j in range(NCC):
                pmx = ptile()
                nc.tensor.transpose(pmx[:, 0:128],
                                    amx_t[:, j * 128:(j + 1) * 128],
                                    ident[:])
                nc.vector.tensor_reduce(Mx[:, j:j + 1], pmx[:, 0:128],
                                        axis=AX.X, op=OP.max)
                pmn = ptile()
                nc.tensor.transpose(pmn[:, 0:128],
                                    amn_t[:, j * 128:(j + 1) * 128],
                                    ident[:])
                nc.vector.tensor_reduce(Mn[:, j:j + 1], pmn[:, 0:128],
                                        axis=AX.X, op=OP.min)
            if beta_zero:
                # amax = max(Mx, -Mn); gamma_q = max(|g| * amax, eps)
                t2 = small.tile([128, NCC], F32, tag="t2", name="t2")
                nc.vector.scalar_tensor_tensor(t2[:], Mn[:], -1.0, Mx[:],
                                               op0=OP.mult, op1=OP.max)
                ga = small.tile([128, NCC], F32, tag="ga", name="ga")
                nc.vector.scalar_tensor_tensor(ga[:], g_mat[:], -1.0,
                                               g_mat[:], op0=OP.mult,
                                               op1=OP.max)
                nc.vector.tensor_tensor(m1[:], ga[:], t2[:], op=OP.mult)
                nc.vector.tensor_scalar_max(m1[:], m1[:], EPS_CLAMP)
            else:
                t1 = small.tile([128, NCC], F32, tag="t1", name="t1")
                t2 = small.tile([128, NCC], F32, tag="t2", name="t2")
                nc.vector.tensor_tensor(t1[:], g_mat[:], Mx[:], op=OP.mult)
                nc.vector.tensor_tensor(t2[:], g_mat[:], Mn[:], op=OP.mult)
                nc.vector.tensor_tensor(t1[:], t1[:], b_mat[:], op=OP.add)
                nc.vector.tensor_tensor(t2[:], t2[:], b_mat[:], op=OP.add)
                nc.vector.tensor_tensor(m1[:], t1[:], t2[:], op=OP.max)
                nc.vector.tensor_scalar_mul(t2[:], t2[:], -1.0)
                nc.vector.tensor_scalar_mul(t1[:], t1[:], -1.0)
                nc.vector.tensor_tensor(m1[:], m1[:], t2[:], op=OP.max)
                nc.vector.tensor_tensor(m1[:], m1[:], t1[:], op=OP.max)
                nc.vector.tensor_scalar_max(m1[:], m1[:], EPS_CLAMP)
            ginv = small.tile([128, NCC], F32, tag="ginv", name="ginv")
            nc.vector.reciprocal(ginv[:], m1[:])
            sc_m = small.tile([128, NCC], F32, tag="scm", name="scm")
            nc.vector.tensor_scalar_mul(sc_m[:], ginv[:], QP)
            scinv = small.tile([128, NCC], F32, tag="sci", name="sci")
            nc.vector.reciprocal(scinv[:], sc_m[:])
            A_m = small.tile([128, NCC], F32, tag="Am", name="Am")
            nc.vector.tensor_tensor(A_m[:], g_mat[:], sc_m[:], op=OP.mult)
            r_m = small.tile([128, NCC], F32, tag="rm", name="rm")
            nc.vector.tensor_scalar_mul(r_m[:], scinv[:], beta_col[:])
            if not beta_zero:
                B_m = small.tile([128, NCC], F32, tag="Bm", name="Bm")
                nc.vector.tensor_tensor(B_m[:], b_mat[:], sc_m[:], op=OP.mult)
            mats = [(A_m, A_b), (r_m, r_b)]
            if not beta_zero:
                mats.append((B_m, B_b))
            for j in range(NCC):
                cs = slice(j * 128, (j + 1) * 128)
                for mat, dst in mats:
                    prow = ptile()
                    nc.tensor.transpose(prow[0:1, 0:128], mat[:, j:j + 1],
                                        ident[:])
                    rw = small.tile([1, 128], F32, tag="rw", name="rw", bufs=2)
                    nc.vector.tensor_copy(rw[:], prow[0:1, 0:128])
                    pbc = ptile()
                    nc.tensor.matmul(pbc[:, 0:128], ones_row[:], rw[:],
                                     start=True, stop=True)
                    nc.vector.tensor_copy(dst[:, cs], pbc[:, 0:128])

            # ============ Pass X2 + transpose + matmul ====================
            # xqt_all allocated after the W stash is released (zone reuse)
            xqt_pool = ctx.enter_context(tc.tile_pool(name="xqt", bufs=1))
            xqt_all = xqt_pool.tile([128, NCC, T + 2 * XPAD], BF16)
            for j in range(NCC):
                nc.vector.memset(xqt_all[:, j, XPAD - 1:XPAD], 0.0)
                nc.vector.memset(xqt_all[:, j, XPAD + T:XPAD + T + 1], 0.0)

            def produce(q):
                for itq in range(NTQ):
                    it = q * NTQ + itq
                    xt = xin2.tile([128, C], F32, tag="xt2", name="xt2")
                    nc.scalar.dma_start(out=xt[:],
                                        in_=x_d[it * 128:(it + 1) * 128, :])
                    xh = xhat_p.tile([128, C], F32, tag="xh2", name="xh2")
                    nc.scalar.activation(xh[:], xt[:], AF.Identity,
                                         bias=nmr_all[:, it:it + 1],
                                         scale=rsig_all[:, it:it + 1])
                    nc.vector.tensor_tensor(xh[:], xh[:], A_b[:], op=OP.mult)
                    if not beta_zero:
                        nc.vector.tensor_tensor(xh[:], xh[:], B_b[:], op=OP.add)
                    # fused round-to-nearest-even: (v + RC) - RC, bf16 out
                    xq = xq_p.tile([128, C], BF16, tag="xq", name="xq")
                    nc.vector.tensor_scalar(xq[:], xh[:], RC, RC,
                                            op0=OP.add, op1=OP.subtract)
                    nc.scalar.dma_start(out=xq_dram[it * 128:(it + 1) * 128, :],
                                        in_=xq[:])
                for j in range(NCC):
                    nc.sync.dma_start_transpose(
                        xqt_all[:, j, XPAD + q * TQ:XPAD + (q + 1) * TQ],
                        xq_dram[q * TQ:(q + 1) * TQ, j * 128:(j + 1) * 128])

            def consume(q):
                for itq in range(NTQ):
                    it = q * NTQ + itq
                    pss = [psum_mm.tile([128, OSL], F32, tag="mm", name="pmm")
                           for _ in range(NH)]
                    for j in range(NCC):
                        for k in range(KW):
                            lhsT = xqt_all[:, j, XPAD + it * 128 + k - 1:
                                           XPAD + it * 128 + k - 1 + 128]
                            first = (j == 0 and k == 0)
                            last = (j == NCC - 1 and k == KW - 1)
                            for h in range(NH):
                                nc.tensor.matmul(
                                    pss[h][:], lhsT,
                                    wqt_all[:, k, j, h * OSL:(h + 1) * OSL],
                                    start=first, stop=last)
                    for h in range(NH):
                        yt = yout.tile([128, OSL], F32, tag="yt", name="yt")
                        nc.vector.tensor_tensor(
                            yt[:], pss[h][:], r_b[:, h * OSL:(h + 1) * OSL],
                            op=OP.mult)
                        nc.gpsimd.dma_start(
                            out=out_d[it * 128:(it + 1) * 128,
                                      h * OSL:(h + 1) * OSL],
                            in_=yt[:])

            for q in range(NQ):
                produce(q)
                if q >= 1:
                    consume(q - 1)
            consume(NQ - 1)

    nc.compile()
    return nc


_NC_CACHE = {}


def _get_nc(T, C, beta_zero):
    key = (T, C, beta_zero)
    if key not in _NC_CACHE:
        _NC_CACHE[key] = build_kernel(T, C, beta_zero)
    return _NC_CACHE[key]


def run(inputs, trace=False):
    """Run the SPMD kernel; returns (output [B,T,C], BassKernelResults)."""
    x = np.ascontiguousarray(np.asarray(inputs["x"], dtype=np.float32))
    g = np.ascontiguousarray(np.asarray(inputs["ln_gamma"], dtype=np.float32))
    b = np.ascontiguousarray(np.asarray(inputs["ln_beta"], dtype=np.float32))
    W = np.ascontiguousarray(np.asarray(inputs["W"], dtype=np.float32))
    B, T, C = x.shape
    assert B == N_CORES, f"expected batch {N_CORES}, got {B}"
    beta_zero = bool(np.all(b == 0.0))
    nc = _get_nc(T, C, beta_zero)
    in_maps = [
        {"x": np.ascontiguousarray(x[i]), "ln_gamma": g, "ln_beta": b, "W": W}
        for i in range(B)
    ]
    res = run_bass_kernel_spmd(nc, in_maps, core_ids=list(range(N_CORES)),
                               trace=trace)
    out = np.stack([res.results[i]["out"] for i in range(B)], axis=0)
    return out, res


def kernel(**inputs) -> np.ndarray:
    out, _ = run(inputs)
    return out


# revision 33
# speedup vs baseline: 1.1064x; 1.1064x over previous
# BitConvBlock Trainium2 kernel: LayerNorm -> activation int8-quant ->
# ternary weight quant -> conv1d(K=3, pad 1) -> rescale.
#
# Sharding: data-parallel over batch (B=8) across the 8 NeuronCores; every
# core gets one batch element plus replicated W / ln params, computes its
# full [T, C] output slice, host stacks the results.
#
# v3 schedule (trace-driven):
#   - prologue: x read once (stats+xhat+extrema, tile held in SBUF); W read
#     once into an SBUF stash. Engine split sized so every queue drains by
#     the DMA floor (~80us): DVE = bn_stats + fp16 extrema + tiny rsqrt
#     chains; ACT = xhat + W abs + W sign; gpsimd = W fma; PE = wq
#     transposes.
#   - main window: produce(q) re-reads x, quantizes (ACT xhat, DVE mult +
#     one fused RNE round), stores bf16 xq to DRAM and XBAR-transposes it
#     back (triggers split across the sync and scalar queues); consume(q)
#     is a pure PE matmul stream with DVE rescale + gpsimd output DMA.
#     Produce-only pools are allocated after the W stash dies so they can
#     be deep enough to prefetch ahead of the PE.
#
# Exactness: x_q integer in [-127,127], w_q in {-1,0,1}; both exact in
# bf16; every partial conv sum < 2^24 so bf16 matmul + fp32 PSUM accum is
# exact. Rounding uses the fp32 +-1.5*2^23 trick (round-to-nearest-even).
# The activation absmax is taken over an fp16 copy of xhat (<=2^-11
# relative error on the quant scale, ~0.4% output L2 vs the 2e-2 gate).

import numpy as np

import concourse.bacc as bacc
import concourse.bass as bass
import concourse.mybir as mybir
import concourse.tile as tile
from concourse.bass_utils import run_bass_kernel_spmd
from concourse.masks import make_identity

F32 = mybir.dt.float32
BF16 = mybir.dt.bfloat16
FP16 = mybir.dt.float16
AX = mybir.AxisListType
OP = mybir.AluOpType
AF = mybir.ActivationFunctionType

QP = 127.0
EPS_LN = 1e-5
EPS_CLAMP = 1e-5
RC = 1.5 * 2.0**23  # fp32 round-to-nearest-even magic constant
N_CORES = 8
KW = 3  # conv kernel width


def build_kernel(T, C, beta_zero, n_cores=N_CORES):
    """Build and compile the per-core Bass program for x:[T,C] W:[C,C,3]."""
    assert T % 128 == 0 and C % 128 == 0
    NT = T // 128            # time tiles
    NCC = C // 128           # channel chunks of 128
    OSL = min(512, C)        # output-channel slab (one PSUM bank)
    NH = C // OSL            # slabs per tile
    TQ = min(512, T)         # produce/consume chunk along T
    NQ = T // TQ
    NTQ = TQ // 128          # time tiles per chunk
    SUB = min(512, C)        # bn_stats subgroup
    NS = C // SUB
    SG = 2                   # stats group (x tiles held in SBUF)
    NG = NT // SG
    XPAD = 16                # left pad in xqT so xbar writes stay 32B-aligned
    W_COUNT = float(C * C * KW)

    nc = bacc.Bacc("TRN2", target_bir_lowering=False, debug=False,
                   num_devices=n_cores)
    x_d = nc.dram_tensor("x", [T, C], F32, kind="ExternalInput")
    g_d = nc.dram_tensor("ln_gamma", [C], F32, kind="ExternalInput")
    b_d = nc.dram_tensor("ln_beta", [C], F32, kind="ExternalInput")
    w_d = nc.dram_tensor("W", [C, C, KW], F32, kind="ExternalInput")
    out_d = nc.dram_tensor("out", [T, C], F32, kind="ExternalOutput")

    with tile.TileContext(nc) as tc:
        import contextlib
        with contextlib.ExitStack() as ctx:
            dram = ctx.enter_context(tc.tile_pool(name="dram", bufs=1, space="DRAM"))
            xq_dram = dram.tile([T, C], BF16)

            const = ctx.enter_context(tc.tile_pool(name="const", bufs=1))
            ident = const.tile([128, 128], F32)
            make_identity(nc, ident[:])
            identb = const.tile([128, 128], BF16)
            nc.vector.tensor_copy(identb[:], ident[:])
            identh = const.tile([128, 128], FP16)
            nc.vector.tensor_copy(identh[:], ident[:])
            ones_col = const.tile([128, 1], F32)
            nc.vector.memset(ones_col[:], 1.0)
            ones_row = const.tile([1, 128], F32)
            nc.vector.memset(ones_row[:], 1.0)
            rcn_col = const.tile([128, 1], F32)
            nc.vector.memset(rcn_col[:], -RC)
            eps_col = const.tile([128, 1], F32)
            nc.vector.memset(eps_col[:], EPS_LN)

            mv_all = const.tile([128, NT, 2], F32)    # per-tile mean/var
            rsig_all = const.tile([128, NT], F32)
            nmr_all = const.tile([128, NT], F32)      # -mu * rsig
            wabs = const.tile([128, NCC], F32)        # per o-tile |W| row sums
            beta_col = const.tile([128, 1], F32)
            binv_col = const.tile([128, 1], F32)

            # running extrema of fp16 xhat, one slot per group member
            amx2 = const.tile([128, SG, C], FP16)
            amn2 = const.tile([128, SG, C], FP16)

            # big persistent bf16 weight operand
            wqt_all = const.tile([128, KW, NCC, C], BF16)

            # ---------------- prologue pools ------------------------------
            xin = ctx.enter_context(tc.tile_pool(name="xin", bufs=4))
            xhat_p = ctx.enter_context(tc.tile_pool(name="xhat", bufs=2))
            wq_p = ctx.enter_context(tc.tile_pool(name="wq", bufs=2))
            small = ctx.enter_context(tc.tile_pool(name="small", bufs=4))
            st_p = ctx.enter_context(tc.tile_pool(name="st", bufs=2))
            grp_p = ctx.enter_context(tc.tile_pool(name="grp", bufs=3))

            psum_mm = ctx.enter_context(
                tc.tile_pool(name="psum_mm", bufs=6, space="PSUM"))
            psum_ms = ctx.enter_context(
                tc.tile_pool(name="psum_ms", bufs=2, space="PSUM"))

            def ptile():
                return psum_ms.tile([128, 512], F32, tag="ms", name="pms")

            def pbtile():
                return psum_ms.tile([128, 512], BF16, tag="ms", name="pmsb")

            def phtile():
                return psum_ms.tile([128, 512], FP16, tag="ms", name="pmsh")

            # =================== prologue =================================
            with tc.tile_pool(name="wstash", bufs=1) as stash_p, \
                 tc.tile_pool(name="upool", bufs=2) as u_pool:
                wstash = stash_p.tile([128, NCC, C, KW], F32)

                # W loads first (scalar HWDGE queue) so W lands early.
                for ot in range(NCC):
                    nc.scalar.dma_start(
                        out=wstash[:, ot, :, :],
                        in_=w_d[ot * 128:(ot + 1) * 128, :, :])

                # |W| row-sums for one o-tile: single contiguous ACT Abs
                # with accumulate; the bf16 |W| values land in a not-yet-
                # written slice of wqt_all (pure scratch, overwritten by
                # the real wq transposes later).
                def wabs_tile(ot):
                    trash = wqt_all[:, 0, 0:KW, :].rearrange(
                        "p a b -> p (a b)")
                    nc.scalar.activation(
                        trash, wstash[:, ot, :, :].rearrange(
                            "p a b -> p (a b)"),
                        AF.Abs, accum_out=wabs[:, ot:ot + 1])

                # beta_w = max(mean|W|, eps), replicated to [128,1]
                def beta_chain():
                    wsum = small.tile([128, 1], F32, tag="wsum", name="wsum")
                    nc.vector.reduce_sum(wsum[:], wabs[:], axis=AX.X)
                    ps1 = psum_ms.tile([1, 1], F32, tag="ms", name="ps1")
                    nc.tensor.matmul(ps1[:], ones_col[:], wsum[:], start=True,
                                     stop=True)
                    bsc = small.tile([1, 1], F32, tag="bsc", name="bsc")
                    nc.vector.tensor_scalar(bsc[:], ps1[:], 1.0 / W_COUNT,
                                            EPS_CLAMP, op0=OP.mult, op1=OP.max)
                    psb = psum_ms.tile([128, 1], F32, tag="ms", name="psb")
                    nc.tensor.matmul(psb[:], ones_row[:], bsc[:], start=True,
                                     stop=True)
                    nc.vector.tensor_copy(beta_col[:], psb[:])
                    nc.vector.reciprocal(binv_col[:], beta_col[:])

                # W quantize + transpose, one o-tile at a time.
                # u = rne(w/beta) + RC (DVE fma with the RC grid trick);
                # Sign(u - RC) on ACT; PE transpose blocks.
                def w2_tile(ot):
                    for k in range(KW):
                        ut = u_pool.tile([128, C], F32, tag="ut", name="ut")
                        nc.vector.tensor_scalar(ut[:], wstash[:, ot, :, k],
                                                binv_col[:], RC,
                                                op0=OP.mult, op1=OP.add)
                        wqk = wq_p.tile([128, C], BF16, tag="wqk", name="wqk")
                        nc.scalar.activation(wqk[:], ut[:], AF.Sign,
                                             bias=rcn_col[:], scale=1.0)
                        for jb2 in range(0, NCC, 4):
                            pb = pbtile()
                            nblk = min(4, NCC - jb2)
                            for b in range(nblk):
                                jb = jb2 + b
                                nc.tensor.transpose(
                                    pb[:, b * 128:(b + 1) * 128],
                                    wqk[:, jb * 128:(jb + 1) * 128],
                                    identb[:])
                            nc.vector.tensor_copy(
                                wqt_all[:, k, jb2:jb2 + nblk,
                                        ot * 128:(ot + 1) * 128],
                                pb[:, 0:nblk * 128])

                # x sweep group g covers tiles [g*SG, (g+1)*SG)
                def x_group(g):
                    xts = []
                    for itq in range(SG):
                        it = g * SG + itq
                        xt = xin.tile([128, C], F32)
                        nc.sync.dma_start(out=xt[:],
                                          in_=x_d[it * 128:(it + 1) * 128, :])
                        st6 = st_p.tile([128, NS, 6], F32)
                        for sb in range(NS):
                            nc.vector.bn_stats(st6[:, sb, :],
                                               xt[:, sb * SUB:(sb + 1) * SUB])
                        nc.vector.bn_aggr(mv_all[:, it, :], st6[:])
                        xts.append(xt)
                    gs = slice(g * SG, (g + 1) * SG)
                    # rsig = 1/sqrt(var + eps) (no Newton step: DVE recip +
                    # ACT sqrt are well within the 2e-2 gate)
                    s0 = grp_p.tile([128, SG], F32, tag="g1", name="gt")
                    nc.scalar.activation(s0[:], mv_all[:, gs, 1], AF.Sqrt,
                                         bias=eps_col[:], scale=1.0)
                    nc.vector.reciprocal(rsig_all[:, gs], s0[:])
                    # nmr = -mu * rsig, one fused op
                    nc.vector.scalar_tensor_tensor(
                        nmr_all[:, gs], mv_all[:, gs, 0], -1.0,
                        rsig_all[:, gs], op0=OP.mult, op1=OP.mult)
                    # fp16 xhat for the extrema chains
                    xhg = xhat_p.tile([128, SG, C], FP16, tag="xhp",
                                      name="xhp")
                    for itq in range(SG):
                        it = g * SG + itq
                        nc.scalar.activation(xhg[:, itq, :], xts[itq][:],
                                             AF.Identity,
                                             bias=nmr_all[:, it:it + 1],
                                             scale=rsig_all[:, it:it + 1])
                    if g == 0:
                        nc.vector.tensor_copy(amx2[:], xhg[:])
                        nc.vector.tensor_copy(amn2[:], xhg[:])
                    else:
                        nc.vector.tensor_tensor(amx2[:], amx2[:], xhg[:],
                                                op=OP.max)
                        nc.vector.tensor_tensor(amn2[:], amn2[:], xhg[:],
                                                op=OP.min)

                # interleave: x groups paced by DMA; W abs / beta / quant
                # slotted between groups to match W DMA arrival.
                wa_done = 0
                beta_done = False
                w2_done = 0
                for g in range(NG):
                    x_group(g)
                    if g == 0:
                        continue
                    if wa_done < NCC:
                        wabs_tile(wa_done)
                        wa_done += 1
                    elif not beta_done:
                        beta_chain()
                        beta_done = True
                    elif w2_done < NCC:
                        for _ in range(2):
                            if w2_done < NCC:
                                w2_tile(w2_done)
                                w2_done += 1
                while wa_done < NCC:
                    wabs_tile(wa_done)
                    wa_done += 1
                if not beta_done:
                    beta_chain()
                while w2_done < NCC:
                    w2_tile(w2_done)
                    w2_done += 1

            # ============ per-channel scales -> broadcast rows ============
            bcast = ctx.enter_context(tc.tile_pool(name="bcast", bufs=1))
            A_b = bcast.tile([128, C], F32)
            B_b = (bcast.tile([128, C], F32, name="B_b") if not beta_zero
                   else None)
            r_b = bcast.tile([128, C], F32)
            g_mat = small.tile([128, NCC], F32, tag="gmat", name="gmat")
            nc.gpsimd.dma_start(out=g_mat[:],
                                in_=g_d.ap().rearrange("(j p) -> p j", p=128))
            if not beta_zero:
                b_mat = small.tile([128, NCC], F32, tag="bmat", name="bmat")
                nc.gpsimd.dma_start(out=b_mat[:],
                                    in_=b_d.ap().rearrange("(j p) -> p j", p=128))
            # combine the SG extrema slots (converting fp16 -> f32 into the
            # not-yet-written A_b / r_b buffers), then per-channel reduce
            nc.vector.tensor_tensor(amx2[:, 0, :], amx2[:, 0, :],
                                    amx2[:, 1, :], op=OP.max)
            nc.vector.tensor_tensor(amn2[:, 0, :], amn2[:, 0, :],
                                    amn2[:, 1, :], op=OP.min)
            nc.vector.tensor_copy(A_b[:], amx2[:, 0, :])
            nc.vector.tensor_copy(r_b[:], amn2[:, 0, :])
            m1 = small.tile([128, NCC], F32, tag="m1", name="m1")
            Mx = small.tile([128, NCC], F32, tag="Mx", name="Mx")
            Mn = small.tile([128, NCC], F32, tag="Mn", name="Mn")
            for j in range(NCC):
                pmx = ptile()
                nc.tensor.transpose(pmx[:, 0:128],
                                    A_b[:, j * 128:(j + 1) * 128],
                                    ident[:])
                nc.vector.tensor_reduce(Mx[:, j:j + 1], pmx[:, 0:128],
                                        axis=AX.X, op=OP.max)
                pmn = ptile()
                nc.tensor.transpose(pmn[:, 0:128],
                                    r_b[:, j * 128:(j + 1) * 128],
                                    ident[:])
                nc.vector.tensor_reduce(Mn[:, j:j + 1], pmn[:, 0:128],
                                        axis=AX.X, op=OP.min)
            if beta_zero:
                # amax = max(Mx, -Mn); gamma_q = max(|g| * amax, eps)
                t2 = small.tile([128, NCC], F32, tag="t2", name="t2")
                nc.vector.scalar_tensor_tensor(t2[:], Mn[:], -1.0, Mx[:],
                                               op0=OP.mult, op1=OP.max)
                ga = small.tile([128, NCC], F32, tag="ga", name="ga")
                nc.vector.scalar_tensor_tensor(ga[:], g_mat[:], -1.0,
                                               g_mat[:], op0=OP.mult,
                                               op1=OP.max)
                nc.vector.tensor_tensor(m1[:], ga[:], t2[:], op=OP.mult)
                nc.vector.tensor_scalar_max(m1[:], m1[:], EPS_CLAMP)
            else:
                t1 = small.tile([128, NCC], F32, tag="t1", name="t1")
                t2 = small.tile([128, NCC], F32, tag="t2", name="t2")
                nc.vector.tensor_tensor(t1[:], g_mat[:], Mx[:], op=OP.mult)
                nc.vector.tensor_tensor(t2[:], g_mat[:], Mn[:], op=OP.mult)
                nc.vector.tensor_tensor(t1[:], t1[:], b_mat[:], op=OP.add)
                nc.vector.tensor_tensor(t2[:], t2[:], b_mat[:], op=OP.add)
                nc.vector.tensor_tensor(m1[:], t1[:], t2[:], op=OP.max)
                nc.vector.tensor_scalar_mul(t2[:], t2[:], -1.0)
                nc.vector.tensor_scalar_mul(t1[:], t1[:], -1.0)
                nc.vector.tensor_tensor(m1[:], m1[:], t2[:], op=OP.max)
                nc.vector.tensor_tensor(m1[:], m1[:], t1[:], op=OP.max)
                nc.vector.tensor_scalar_max(m1[:], m1[:], EPS_CLAMP)
            ginv = small.tile([128, NCC], F32, tag="ginv", name="ginv")
            nc.vector.reciprocal(ginv[:], m1[:])
            sc_m = small.tile([128, NCC], F32, tag="scm", name="scm")
            nc.vector.tensor_scalar_mul(sc_m[:], ginv[:], QP)
            scinv = small.tile([128, NCC], F32, tag="sci", name="sci")
            nc.vector.reciprocal(scinv[:], sc_m[:])
            A_m = small.tile([128, NCC], F32, tag="Am", name="Am")
            nc.vector.tensor_tensor(A_m[:], g_mat[:], sc_m[:], op=OP.mult)
            r_m = small.tile([128, NCC], F32, tag="rm", name="rm")
            nc.vector.tensor_scalar_mul(r_m[:], scinv[:], beta_col[:])
            if not beta_zero:
                B_m = small.tile([128, NCC], F32, tag="Bm", name="Bm")
                nc.vector.tensor_tensor(B_m[:], b_mat[:], sc_m[:], op=OP.mult)
            # broadcast columns to [128,128] blocks; A_b first (it gates
            # the produce quantization), r_b/B_b after (only gate rescale)
            mats = [(A_m, A_b), (r_m, r_b)]
            if not beta_zero:
                mats.insert(1, (B_m, B_b))
            for mat, dst in mats:
                for j in range(NCC):
                    cs = slice(j * 128, (j + 1) * 128)
                    prow = ptile()
                    nc.tensor.transpose(prow[0:1, 0:128], mat[:, j:j + 1],
                                        ident[:])
                    rw = small.tile([1, 128], F32, tag="rw", name="rw", bufs=2)
                    nc.vector.tensor_copy(rw[:], prow[0:1, 0:128])
                    pbc = ptile()
                    nc.tensor.matmul(pbc[:, 0:128], ones_row[:], rw[:],
                                     start=True, stop=True)
                    nc.vector.tensor_copy(dst[:, cs], pbc[:, 0:128])

            # ============ produce/consume pools (post-stash) ==============
            xin2 = ctx.enter_context(tc.tile_pool(name="xin2", bufs=5))
            xq_p = ctx.enter_context(tc.tile_pool(name="xq", bufs=3))
            yout = ctx.enter_context(tc.tile_pool(name="yout", bufs=4))
            xqt_pool = ctx.enter_context(tc.tile_pool(name="xqt", bufs=1))
            xqt_all = xqt_pool.tile([128, NCC, T + 2 * XPAD], BF16)
            for j in range(NCC):
                nc.vector.memset(xqt_all[:, j, XPAD - 1:XPAD], 0.0)
                nc.vector.memset(xqt_all[:, j, XPAD + T:XPAD + T + 1], 0.0)

            def produce(q):
                for itq in range(NTQ):
                    it = q * NTQ + itq
                    xt = xin2.tile([128, C], F32, tag="xt2", name="xt2")
                    nc.scalar.dma_start(out=xt[:],
                                        in_=x_d[it * 128:(it + 1) * 128, :])
                    xh = xhat_p.tile([128, C], F32, tag="xh2", name="xh2")
                    # xhat on DVE (2 per-partition AP scalars) to keep the
                    # produce chain off the scalar engine's ACT backlog
                    nc.vector.tensor_scalar(xh[:], xt[:],
                                            rsig_all[:, it:it + 1],
                                            nmr_all[:, it:it + 1],
                                            op0=OP.mult, op1=OP.add)
                    nc.vector.tensor_tensor(xh[:], xh[:], A_b[:], op=OP.mult)
                    if not beta_zero:
                        nc.vector.tensor_tensor(xh[:], xh[:], B_b[:], op=OP.add)
                    # fused round-to-nearest-even: (v + RC) - RC, bf16 out
                    xq = xq_p.tile([128, C], BF16, tag="xq", name="xq")
                    nc.vector.tensor_scalar(xq[:], xh[:], RC, RC,
                                            op0=OP.add, op1=OP.subtract)
                    nc.scalar.dma_start(out=xq_dram[it * 128:(it + 1) * 128, :],
                                        in_=xq[:])
                # XBAR transposes -- all on the sync queue: a transpose
                # issued on the same engine as the xq stores does NOT
                # order against them (different hardware path) and races
                for j in range(NCC):
                    nc.sync.dma_start_transpose(
                        xqt_all[:, j, XPAD + q * TQ:XPAD + (q + 1) * TQ],
                        xq_dram[q * TQ:(q + 1) * TQ, j * 128:(j + 1) * 128])

            def consume(q):
                for itq in range(NTQ):
                    it = q * NTQ + itq
                    pss = [psum_mm.tile([128, OSL], F32, tag="mm", name="pmm")
                           for _ in range(NH)]
                    for j in range(NCC):
                        for k in range(KW):
                            lhsT = xqt_all[:, j, XPAD + it * 128 + k - 1:
                                           XPAD + it * 128 + k - 1 + 128]
                            first = (j == 0 and k == 0)
                            last = (j == NCC - 1 and k == KW - 1)
                            for h in range(NH):
                                nc.tensor.matmul(
                                    pss[h][:], lhsT,
                                    wqt_all[:, k, j, h * OSL:(h + 1) * OSL],
                                    start=first, stop=last)
                    for h in range(NH):
                        yt = yout.tile([128, OSL], F32, tag="yt", name="yt")
                        nc.vector.tensor_tensor(
                            yt[:], pss[h][:], r_b[:, h * OSL:(h + 1) * OSL],
                            op=OP.mult)
                        nc.gpsimd.dma_start(
                            out=out_d[it * 128:(it + 1) * 128,
                                      h * OSL:(h + 1) * OSL],
                            in_=yt[:])

            for q in range(NQ):
                produce(q)
                if q >= 1:
                    consume(q - 1)
            consume(NQ - 1)

    nc.compile()
    return nc


_NC_CACHE = {}


def _get_nc(T, C, beta_zero):
    key = (T, C, beta_zero)
    if key not in _NC_CACHE:
        _NC_CACHE[key] = build_kernel(T, C, beta_zero)
    return _NC_CACHE[key]


def run(inputs, trace=False):
    """Run the SPMD kernel; returns (output [B,T,C], BassKernelResults)."""
    x = np.ascontiguousarray(np.asarray(inputs["x"], dtype=np.float32))
    g = np.ascontiguousarray(np.asarray(inputs["ln_gamma"], dtype=np.float32))
    b = np.ascontiguousarray(np.asarray(inputs["ln_beta"], dtype=np.float32))
    W = np.ascontiguousarray(np.asarray(inputs["W"], dtype=np.float32))
    B, T, C = x.shape
    assert B == N_CORES, f"expected batch {N_CORES}, got {B}"
    beta_zero = bool(np.all(b == 0.0))
    nc = _get_nc(T, C, beta_zero)
    in_maps = [
        {"x": np.ascontiguousarray(x[i]), "ln_gamma": g, "ln_beta": b, "W": W}
        for i in range(B)
    ]
    res = run_bass_kernel_spmd(nc, in_maps, core_ids=list(range(N_CORES)),
                               trace=trace)
    out = np.stack([res.results[i]["out"] for i in range(B)], axis=0)
    return out, res


def kernel(**inputs) -> np.ndarray:
    out, _ = run(inputs)
    return out
